# revision 44
# baseline (speedup 1.0000x reference)
"""Trainium2 Bass kernel for nn_Attention_13864154431876.

Dense transformer attention block: QKV projection + RoPE + causal GQA
attention (32 q heads, 8 kv heads, head_dim 128) + output projection.
B=2, S=2048, D=4096, start_pos=0 (cache fully overwritten).

Sharding (8 NeuronCores, tensor parallel by attention heads):
  - each core owns 4 q-heads and 1 kv-head (wq/wk/wv output-dim shards)
  - x is replicated (shipped pre-transposed as x^T so the contraction dim
    lands on partitions)
  - after attention, an on-chip AllToAll redistributes attn^T from
    head-sharded to token-sharded; each core then multiplies its 512-token
    slab against the full wo and the host concatenates the 8 slabs.

All on-chip data is bf16 (PSUM accumulation stays fp32); the 2e-2
relative-error budget has ample headroom for it, it halves all HBM/DMA
traffic and doubles DVE elementwise throughput.

Attention emission is software-pipelined: the AV matmul for chunk jc
is emitted one step after its score matmul so the exp (scalar engine)
latency never stalls the in-order tensor engine. The causal mask is
applied as a bias-accumulate matmul on the tensor engine (a triangular
-30000 tile added to the diagonal 128x128 sub-block of the scores) and
diagonal score/AV matmuls are shrunk to the live query subrange.
"""
import sys

sys.path.insert(0, "/root/.axon_site/_ro/trn_rl_repo")

import numpy as np
import ml_dtypes

import concourse.bass as bass
import concourse.mybir as mybir
import concourse.tile as tile
from concourse import bacc
from concourse.bass_utils import run_bass_kernel_spmd

F32 = mybir.dt.float32
BF16 = mybir.dt.bfloat16
AF = mybir.ActivationFunctionType
ALU = mybir.AluOpType

N_CORES = 8
B, S, D = 2, 2048, 4096
H, KH, HD = 32, 8, 128
MS = 2048                     # max_seq_len (cache length)
BS = B * S                    # flattened tokens, b-major
HPC = H // N_CORES            # q-heads per core = 4
QF = HPC * HD                 # per-core q-feature width = 512
TB = 512                      # token block
NTB = BS // TB                # 8 token blocks
QBPB = S // TB                # 4 q-blocks per batch element
KC = D // 128                 # 32 contraction chunks
JCB = S // 128                # 16 j-chunks per batch element
SCALE = 1.0 / np.sqrt(HD)
TOKS_PER_CORE = BS // N_CORES  # 512

BF16NP = ml_dtypes.bfloat16


def build_attn_nc(mock_collectives=False):
    nc = bacc.Bacc("TRN2", target_bir_lowering=False, debug=False,
                   num_devices=N_CORES)

    # ---- DRAM I/O ----------------------------------------------------
    xt_d = nc.dram_tensor("xt", [D, BS], BF16, kind="ExternalInput").ap()
    wq_d = nc.dram_tensor("wq", [D, QF], BF16, kind="ExternalInput").ap()
    wk_d = nc.dram_tensor("wk", [D, HD], BF16, kind="ExternalInput").ap()
    wv_d = nc.dram_tensor("wv", [D, HD], BF16, kind="ExternalInput").ap()
    wo_d = nc.dram_tensor("wo", [D, D], BF16, kind="ExternalInput").ap()
    cos_d = nc.dram_tensor("cosT", [HD, S], BF16, kind="ExternalInput").ap()
    sin_d = nc.dram_tensor("sinT", [HD, S], BF16, kind="ExternalInput").ap()
    maskb_d = nc.dram_tensor("maskb", [128, 128], BF16, kind="ExternalInput").ap()
    rot_d = nc.dram_tensor("rotm", [HD, HD], BF16, kind="ExternalInput").ap()
    ident_d = nc.dram_tensor("ident", [128, 128], BF16, kind="ExternalInput").ap()
    ones_d = nc.dram_tensor("ones128", [128, 128], BF16, kind="ExternalInput").ap()
    y_d = nc.dram_tensor("y", [TOKS_PER_CORE, D], F32, kind="ExternalOutput").ap()

    # internal DRAM for the per-batch-element AllToAlls.
    # attn_locX rows are chunk-major: chunk j (512 rows) = my 512 head-feats
    # for token group j. After A2A, attn_gX rows are global head-feats for
    # MY token slab.
    HTB = TB // 2  # 256
    QTB = TB // 4  # 128
    attn_loc = [nc.dram_tensor("attn_loc0", [BS, HTB], BF16),
                nc.dram_tensor("attn_loc1a", [BS, QTB], BF16),
                nc.dram_tensor("attn_loc1b", [BS, QTB], BF16)]
    attn_g = [nc.dram_tensor("attn_g0", [D, HTB], BF16),
              nc.dram_tensor("attn_g1a", [D, QTB], BF16),
              nc.dram_tensor("attn_g1b", [D, QTB], BF16)]

    with tile.TileContext(nc) as tc:
        # ---- pool stack (bottom-up; closed LIFO) --------------------
        persist0_cm = tc.tile_pool(name="persist0", bufs=1)
        persist0 = persist0_cm.__enter__()
        maskb_sb = persist0.tile([128, 128], BF16, name="maskb_sb")
        rot_sb = persist0.tile([HD, HD], BF16, name="rot_sb")
        ident_sb = persist0.tile([128, 128], BF16, name="ident_sb")
        ones_sb = persist0.tile([128, 128], BF16, name="ones_sb")
        kt_sb = [persist0.tile([HD, S], BF16, name=f"kt{b}_sb") for b in range(B)]
        v_sb = [persist0.tile([128, JCB, HD], BF16, name=f"v{b}_sb")
                for b in range(B)]
        attn_sb = [persist0.tile([128, KC, 128], BF16, name=f"attn_sb{i}")
                   for i in range(4)]

        # wo(0) ring: slots exist from the start; the DMAs are emitted just
        # before the tail so the load runs under the tail attention.
        wop_cm = tc.tile_pool(name="wop", bufs=8)
        wop = wop_cm.__enter__()

        tier2 = []

        def pool_t2(*a, **kw):
            cm = tc.tile_pool(*a, **kw)
            p = cm.__enter__()
            tier2.append(cm)
            return p

        qtp = pool_t2(name="qtp", bufs=5)
        tmpp = pool_t2(name="tmpp", bufs=2)
        ptp = pool_t2(name="ptp", bufs=4)
        denp = pool_t2(name="denp", bufs=2)
        recbp = pool_t2(name="recbp", bufs=2)
        atp = pool_t2(name="atp", bufs=2)
        asbp = pool_t2(name="asbp", bufs=2)

        persistA_cm = tc.tile_pool(name="persistA", bufs=1)
        persistA = persistA_cm.__enter__()
        wq_sb = persistA.tile([128, KC, QF], BF16, name="wq_sb")
        wk_sb = persistA.tile([128, KC, HD], BF16, name="wk_sb")
        wv_sb = persistA.tile([128, KC, HD], BF16, name="wv_sb")
        cos_sb = persistA.tile([HD, S], BF16, name="cos_sb")
        sin_sb = persistA.tile([HD, S], BF16, name="sin_sb")
        # weight/constant prefetch rides the scalar-engine DMA queue so it
        # never head-of-line-blocks the latency-critical xt stream (sync
        # queue); chunked so the first projection matmuls start immediately.
        wq_r = wq_d.rearrange("(kc p) n -> p kc n", p=128)
        wk_r = wk_d.rearrange("(kc p) n -> p kc n", p=128)
        wv_r = wv_d.rearrange("(kc p) n -> p kc n", p=128)
        def load_w_group(g4):
            lo, hi = g4 * 4, g4 * 4 + 4
            nc.sync.dma_start(wq_sb[:, lo:hi, :], wq_r[:, lo:hi, :])
            nc.sync.dma_start(wk_sb[:, lo:hi, :], wk_r[:, lo:hi, :])
            nc.sync.dma_start(wv_sb[:, lo:hi, :], wv_r[:, lo:hi, :])

        # weights ride the sync queue interleaved with block 0's xt chunks
        # (emitted inside the kc loop) so FIFO order gives them priority
        # over the deep xt prefetch burst
        load_w_group(0)
        nc.scalar.dma_start(rot_sb[:], rot_d[:])
        nc.scalar.dma_start(ident_sb[:], ident_d[:])
        nc.scalar.dma_start(cos_sb[:], cos_d[:])
        nc.scalar.dma_start(sin_sb[:], sin_d[:])
        nc.scalar.dma_start(maskb_sb[:], maskb_d[:])
        nc.scalar.dma_start(ones_sb[:], ones_d[:])

        trans = []

        def pool_tr(*a, **kw):
            cm = tc.tile_pool(*a, **kw)
            p = cm.__enter__()
            trans.append(cm)
            return p

        # deep ring: the A2A collectives steal DMA bandwidth for ~30us
        # bursts; 12 chunks of lookahead lets the xt stream ride through
        xtp = pool_tr(name="xtp", bufs=12)
        qrawp = pool_tr(name="qrawp", bufs=6)
        vtrawp = pool_tr(name="vtrawp", bufs=2)
        pp_cm = tc.tile_pool(name="pp", bufs=6, space="PSUM")
        pp = pp_cm.__enter__()
        ps_cm = tc.tile_pool(name="ps", bufs=1, space="PSUM")
        ps = ps_cm.__enter__()
        pa_cm = tc.tile_pool(name="pa", bufs=1, space="PSUM")
        pa = pa_cm.__enter__()

        def emit_attention(tb, ps_pool, pa_pool, qt_tiles, lag=1):
            """Generator: pipelined attention for token block tb.

            Per step: score(jc) [+mask bias], exp(jc), den(jc), then the AV
            matmul for chunk jc-lag — so the tensor engine never waits on
            the scalar-engine exp chain."""
            b, qb = tb // QBPB, tb % QBPB
            njc = (qb + 1) * 4
            grp = 0 if tb < QBPB else (1 if tb < 6 else 2)
            for h in range(HPC):
                denacc = denp.tile([128, TB], BF16, name="denacc", tag="den")
                aps = pa_pool.tile([128, TB], F32, name="aps", tag="aps")
                pending = []

                def emit_av(jc, pt, lo):
                    nc.tensor.matmul(
                        aps[:, lo:], v_sb[b][:, jc, :], pt[:, lo:],
                        start=(jc == 0), stop=(jc == njc - 1),
                        skip_group_check=True)

                for jc in range(njc):
                    r = jc - qb * 4
                    lo = max(r, 0) * 128
                    sps = ps_pool.tile([128, TB], F32, name="sps", tag="sps")
                    nc.tensor.matmul(
                        sps[:, lo:], kt_sb[b][:, jc * 128:(jc + 1) * 128],
                        qt_tiles[h][:, lo:], start=True, stop=(r < 0),
                        skip_group_check=True)
                    if r >= 0:
                        # causal bias: triangular -30000 on the diagonal
                        # 128x128 sub-block, accumulated on the PE
                        nc.tensor.matmul(
                            sps[:, lo:lo + 128], ident_sb[:], maskb_sb[:],
                            start=False, stop=True, skip_group_check=True)
                    pt = ptp.tile([128, TB], BF16, name="pt", tag="pt")
                    nc.scalar.activation(pt[:, lo:], sps[:, lo:], AF.Exp)
                    if jc == 0:
                        nc.vector.tensor_copy(denacc[:], pt[:])
                    else:
                        nc.vector.tensor_tensor(denacc[:, lo:], denacc[:, lo:],
                                                pt[:, lo:], ALU.add)
                    pending.append((jc, pt, lo))
                    while len(pending) > lag:
                        emit_av(*pending.pop(0))
                    yield
                while pending:
                    emit_av(*pending.pop(0))
                    yield
                # epilogue: colsum+broadcast via ones-matmul, fast
                # reciprocal, normalize; aps drained by the scalar engine
                asb = asbp.tile([128, TB], BF16, name="asb", tag="asb")
                nc.scalar.copy(asb[:], aps[:])
                denb = ps_pool.tile([128, TB], F32, name="denb", tag="sps")
                nc.tensor.matmul(denb[:], ones_sb[:], denacc[:],
                                 start=True, stop=True, skip_group_check=True)
                recipb = recbp.tile([128, TB], F32, name="recipb", tag="recb")
                nc.vector.reciprocal_approx_fast(recipb[:], denb[:])
                yield
                attn_t = atp.tile([128, TB], BF16, name="attn_t", tag="attn_t")
                nc.vector.tensor_tensor(attn_t[:], asb[:], recipb[:], ALU.mult)
                if grp == 0:
                    for half in range(2):
                        nc.sync.dma_start(
                            attn_loc[0].ap()[
                                (2 * tb + half) * 512 + h * 128:
                                (2 * tb + half) * 512 + (h + 1) * 128, :],
                            attn_t[:, half * HTB:(half + 1) * HTB])
                else:
                    lb = (tb - 4) % 2
                    for qt4 in range(4):
                        nc.sync.dma_start(
                            attn_loc[grp].ap()[
                                (4 * lb + qt4) * 512 + h * 128:
                                (4 * lb + qt4) * 512 + (h + 1) * 128, :],
                            attn_t[:, qt4 * QTB:(qt4 + 1) * QTB])
                yield

        def drive(gen, n):
            if gen is None:
                return None
            for _ in range(n):
                try:
                    next(gen)
                except StopIteration:
                    return None
            return gen

        def emit_a2a(g):
            if mock_collectives:
                nc.sync.dma_start(attn_g[g].ap()[:], attn_loc[g].ap()[:])
            else:
                nc.gpsimd.collective_compute(
                    "AllToAll", ALU.bypass,
                    replica_groups=[list(range(N_CORES))],
                    ins=[attn_loc[g].ap().opt()],
                    outs=[attn_g[g].ap().opt()],
                )

        def load_attn_sb(i):
            if i < 2:
                src = attn_g[0].ap().rearrange("(hc p) q -> p hc q", p=128)[
                    :, :, i * 128:(i + 1) * 128]
            else:
                src = attn_g[i - 1].ap().rearrange("(hc p) q -> p hc q", p=128)
            # SWDGE on the idle Pool engine: these loads depend on the
            # collectives, and on a HWDGE queue they would head-of-line
            # block the latency-critical xt/epilogue/y stream.
            nc.gpsimd.dma_start(attn_sb[i][:], src)

        wo_r = wo_d.rearrange("(hc p) n -> p hc n", p=128)

        def load_wo(ob, pool=None):
            tiles = []
            for g in range(8):
                wt = (pool or wop).tile([128, 4, TB], BF16, name="wo_t",
                                        tag="wo")
                nc.sync.dma_start(
                    wt[:], wo_r[:, g * 4:(g + 1) * 4, ob * TB:(ob + 1) * TB])
                tiles.append(wt)
            return tiles

        prev_gen = None
        prev_steps = 0
        for tb in range(NTB):
            b, qb = tb // QBPB, tb % QBPB
            s0 = qb * TB
            # fractional pacing: spread attention(tb-1)'s steps over the
            # 32 kc iterations AND the drain section so the tensor engine
            # is never left without interleaved work
            pace_slots = KC + 5
            pace_acc = 0
            if tb == NTB - 1:
                # both A2As are long done; load early so the attn_sb tiles
                # are resident before the tail/phase-3 interleave begins
                load_attn_sb(0)
                load_attn_sb(1)
                load_attn_sb(2)
            # ---- projections for tb, interleaved with attention(tb-1)
            qps = [pp.tile([128, TB], F32, name=f"qps{h}", tag="proj")
                   for h in range(HPC)]
            kps = pp.tile([128, TB], F32, name="kps", tag="proj")
            vtps = pp.tile([128, TB], F32, name="vtps", tag="proj")
            for kc in range(KC):
                xt_t = xtp.tile([128, TB], BF16, name="xt_t", tag="xt")
                nc.sync.dma_start(
                    xt_t[:], xt_d[kc * 128:(kc + 1) * 128,
                                  tb * TB:(tb + 1) * TB])
                for h in range(HPC):
                    nc.tensor.matmul(
                        qps[h][:], wq_sb[:, kc, h * 128:(h + 1) * 128],
                        xt_t[:], start=(kc == 0), stop=(kc == KC - 1),
                        skip_group_check=True)
                nc.tensor.matmul(kps[:], wk_sb[:, kc, :], xt_t[:],
                                 start=(kc == 0), stop=(kc == KC - 1),
                                 skip_group_check=True)
                nc.tensor.matmul(vtps[:], wv_sb[:, kc, :], xt_t[:],
                                 start=(kc == 0), stop=(kc == KC - 1),
                                 skip_group_check=True)
                if tb == 0 and kc in (1, 4, 8, 12, 16, 20, 24):
                    load_w_group(kc // 4 + 1)
                tgt = (kc + 1) * prev_steps // pace_slots
                prev_gen = drive(prev_gen, tgt - pace_acc)
                pace_acc = tgt

            # ---- drains + RoPE + V transpose ------------------------
            # All PSUM->SBUF drains go out on the scalar engine first;
            # the rot matmuls then land in the just-freed qps/kps/vtps
            # PSUM banks (pp pool rotation lines up 1:1) so nothing
            # ping-pongs on the attention score bank. K and V complete
            # first so this block's attention can start as soon as its
            # first q head is rotated — filling the drain section when
            # the previous block's attention has run dry.
            kraw = qrawp.tile([128, TB], BF16, name="kraw", tag="qraw")
            nc.scalar.copy(kraw[:], kps[:])
            vtraw = vtrawp.tile([128, TB], BF16, name="vtraw", tag="vtraw")
            nc.scalar.copy(vtraw[:], vtps[:])
            qraws = []
            for h in range(HPC):
                qraw = qrawp.tile([128, TB], BF16, name="qraw", tag="qraw")
                nc.scalar.copy(qraw[:], qps[h][:])
                qraws.append(qraw)
            # K
            rotps = pp.tile([128, TB], F32, name="rotpsk", tag="proj")
            nc.tensor.matmul(rotps[:], rot_sb[:], kraw[:],
                             start=True, stop=True, skip_group_check=True)
            tcos = tmpp.tile([128, TB], BF16, name="tcosk", tag="tmp")
            nc.vector.tensor_tensor(tcos[:], kraw[:],
                                    cos_sb[:, s0:s0 + TB], ALU.mult)
            tsin = tmpp.tile([128, TB], BF16, name="tsink", tag="tmp")
            nc.vector.tensor_tensor(tsin[:], rotps[:],
                                    sin_sb[:, s0:s0 + TB], ALU.mult)
            nc.vector.tensor_tensor(kt_sb[b][:, s0:s0 + TB], tcos[:],
                                    tsin[:], ALU.add)
            # V: transpose 4x [128,128]
            vtr = pp.tile([128, TB], BF16, name="vtr", tag="proj")
            for t4 in range(4):
                nc.tensor.transpose(vtr[:, t4 * 128:(t4 + 1) * 128],
                                    vtraw[:, t4 * 128:(t4 + 1) * 128],
                                    ident_sb[:])
            nc.vector.tensor_copy(
                v_sb[b].rearrange("p jc d -> p (jc d)")[:, s0:s0 + TB],
                vtr[:])
            # Q heads + early start of this block's attention
            qt_tiles = []
            cur_gen = None
            early_steps = 0
            for h in range(HPC):
                rotps = pp.tile([128, TB], F32, name="rotps", tag="proj")
                nc.tensor.matmul(rotps[:], rot_sb[:], qraws[h][:],
                                 start=True, stop=True, skip_group_check=True)
                tcos = tmpp.tile([128, TB], BF16, name="tcos", tag="tmp")
                nc.vector.tensor_tensor(tcos[:], qraws[h][:],
                                        cos_sb[:, s0:s0 + TB], ALU.mult)
                tsin = tmpp.tile([128, TB], BF16, name="tsin", tag="tmp")
                nc.vector.tensor_tensor(tsin[:], rotps[:],
                                        sin_sb[:, s0:s0 + TB], ALU.mult)
                qt = qtp.tile([128, TB], BF16, name="qt", tag="qt")
                nc.vector.tensor_tensor(qt[:], tcos[:], tsin[:], ALU.add)
                qt_tiles.append(qt)
                if tb < NTB - 1 and h == 0:
                    cur_gen = emit_attention(tb, ps, pa, qt_tiles)
                tgt = (KC + 1 + h) * prev_steps // pace_slots
                prev_gen = drive(prev_gen, tgt - pace_acc)
                pace_acc = tgt
            prev_gen = drive(prev_gen, 10 ** 9)  # flush any leftovers
            if tb == 0:
                # block 0 has no previous attention to interleave; fill the
                # drain hole with its own first attention steps (inputs —
                # kt, v, qt0/qt1 — are all complete by this point)
                cur_gen = drive(cur_gen, 8)
                early_steps = 8
            # A2As fire as soon as their group's epilogue DMAs land; the
            # attn_sb loads are EMITTED a full block later so their DMA-
            # completion semaphores (which alias with the xt stream's)
            # never transitively gate projection matmuls on the collective.
            if tb == 4:
                emit_a2a(0)
            elif tb == 6:
                emit_a2a(1)
            if tb < NTB - 1:
                prev_gen = cur_gen
                prev_steps = max(1, HPC * ((qb + 1) * 4 + 3) - early_steps)
            else:
                tail_qt = qt_tiles

        # ---- free projection-only pools; prefetch wo(ob=0) ----------
        wo_next = load_wo(0)
        pa_cm.__exit__(None, None, None)
        ps_cm.__exit__(None, None, None)
        pp_cm.__exit__(None, None, None)
        for cm in reversed(trans):
            cm.__exit__(None, None, None)
        persistA_cm.__exit__(None, None, None)

        # ---- attention tail (tb=7), interleaved with ob=0's output
        # matmuls (dependency-free: attn_sb[0..2] + wo(0) are resident),
        # which fill the tensor-engine bubbles of the exp-bound tail ----
        py_cm = tc.tile_pool(name="py", bufs=4, space="PSUM")
        pyp = py_cm.__enter__()
        ps2_cm = tc.tile_pool(name="ps2", bufs=3, space="PSUM")
        ps2 = ps2_cm.__enter__()
        pa2_cm = tc.tile_pool(name="pa2", bufs=1, space="PSUM")
        pa2 = pa2_cm.__enter__()

        def ygroup_gen(ob, tc4, wo_g):
            yps = pyp.tile([128, TB], F32, name="yps", tag="yps")
            for hc in range(KC):
                nc.tensor.matmul(
                    yps[:], attn_sb[tc4][:, hc, :],
                    wo_g[hc // 4][:, hc % 4, :],
                    start=(hc == 0), stop=(hc == KC - 1),
                    skip_group_check=True)
                if hc % 4 == 3:
                    yield
            y_sb = persist0.tile([128, TB], F32, name="y_sb", tag="y",
                                 bufs=6)
            nc.vector.tensor_copy(y_sb[:], yps[:])
            nc.sync.dma_start(
                y_d[tc4 * 128:(tc4 + 1) * 128,
                    ob * TB:(ob + 1) * TB], y_sb[:])

        def emit_ygroup(ob, tc4, wo_g):
            drive(ygroup_gen(ob, tc4, wo_g), 10 ** 9)

        def chain_gens(gens):
            for g in gens:
                yield from g

        tail_gen = emit_attention(NTB - 1, ps2, pa2, tail_qt, lag=2)
        ygen = chain_gens([ygroup_gen(0, t, wo_next) for t in range(3)])
        cnt = 0
        while tail_gen is not None:
            tail_gen = drive(tail_gen, 1)
            cnt += 1
            if cnt % 3 == 0:
                ygen = drive(ygen, 1)
        emit_a2a(2)
        ygen = drive(ygen, 10 ** 9)
        pa2_cm.__exit__(None, None, None)
        ps2_cm.__exit__(None, None, None)
        for cm in reversed(tier2):
            cm.__exit__(None, None, None)

        wop2_cm = tc.tile_pool(name="wop2", bufs=24)
        wop2 = wop2_cm.__enter__()

        # ---- phase 3 remainder --------------------------------------
        # The tc4=3 groups need attn_sb[3] which arrives only after the
        # final A2A (~30us) — so ob1/ob2's tc4 0..2 groups (~40us of
        # matmuls) are emitted first as cover, with the t3 load deferred
        # past them so its DMA semaphore gates nothing early.
        wo_t = {0: wo_next}
        wo_t[1] = load_wo(1, wop2)
        wo_t[2] = load_wo(2, wop2)
        wo_t[3] = load_wo(3, wop2)
        for tc4 in range(3):
            emit_ygroup(1, tc4, wo_t[1])
        for tc4 in range(3):
            emit_ygroup(2, tc4, wo_t[2])
        load_attn_sb(3)
        emit_ygroup(0, 3, wo_t[0])
        emit_ygroup(1, 3, wo_t[1])
        wo_t[4] = load_wo(4, wop2)
        emit_ygroup(2, 3, wo_t[2])
        wo_t[5] = load_wo(5, wop2)
        for tc4 in range(4):
            emit_ygroup(3, tc4, wo_t[3])
        wo_t[6] = load_wo(6, wop2)
        for tc4 in range(4):
            emit_ygroup(4, tc4, wo_t[4])
        wo_t[7] = load_wo(7, wop2)
        for ob in range(5, 8):
            for tc4 in range(4):
                if ob == 7 and tc4 == 3:
                    continue
                emit_ygroup(ob, tc4, wo_t[ob])
        # final group in two column halves so the first half's drain and
        # store overlap the second half's matmuls
        for half in range(2):
            yps = pyp.tile([128, HTB], F32, name="ypsh", tag="yps")
            for hc in range(KC):
                nc.tensor.matmul(
                    yps[:], attn_sb[3][:, hc, :],
                    wo_t[7][hc // 4][:, hc % 4,
                                     half * HTB:(half + 1) * HTB],
                    start=(hc == 0), stop=(hc == KC - 1),
                    skip_group_check=True)
            y_sb = persist0.tile([128, HTB], F32, name="y_sbh", tag="yh",
                                 bufs=2)
            nc.vector.tensor_copy(y_sb[:], yps[:])
            nc.sync.dma_start(
                y_d[3 * 128:4 * 128,
                    7 * TB + half * HTB:7 * TB + (half + 1) * HTB], y_sb[:])
        py_cm.__exit__(None, None, None)
        wop2_cm.__exit__(None, None, None)
        wop_cm.__exit__(None, None, None)
        persist0_cm.__exit__(None, None, None)

    nc.compile()
    return nc


_NC_CACHE = None


def _get_nc():
    global _NC_CACHE
    if _NC_CACHE is None:
        _NC_CACHE = build_attn_nc()
    return _NC_CACHE


def _host_reference(x, wq, wk, wv, wo, sincos, start_pos, causal_mask):
    """Numpy fallback (only used if the mask is not causal-tril)."""
    xq = (x @ wq).reshape(B, S, H, HD)
    xk = (x @ wk).reshape(B, S, KH, HD)
    xv = (x @ wv).reshape(B, S, KH, HD)
    sp = min(max(int(start_pos), 0), MS - S)
    sc = sincos[sp:sp + S]
    sin, cos = sc[:, :HD], sc[:, HD:]
    sin = sin[None, :, None, :]
    cos = cos[None, :, None, :]

    def rot(u):
        return np.concatenate([-u[..., HD // 2:], u[..., :HD // 2]], axis=-1)

    xq = xq * cos + rot(xq) * sin
    xk = xk * cos + rot(xk) * sin
    mask = np.broadcast_to(causal_mask[:, sp:sp + S, :MS], (B, S, MS))
    out = np.zeros((B, S, H, HD), dtype=np.float32)
    nrep = H // KH
    for b in range(B):
        for h in range(H):
            q = xq[b, :, h]
            k = xk[b, :, h // nrep]
            v = xv[b, :, h // nrep]
            s = (q @ k.T) * SCALE
            s = np.where(mask[b], s, -np.inf)
            s = s - s.max(axis=-1, keepdims=True)
            p = np.exp(s)
            p /= p.sum(axis=-1, keepdims=True)
            out[b, :, h] = p @ v
    return out.reshape(B, S, H * HD) @ wo


def kernel(x, wq, wk, wv, wo, cache_k, cache_v, sincos, causal_mask,
           start_pos):
    x = np.asarray(x, dtype=np.float32)
    wq = np.asarray(wq, dtype=np.float32)
    wk = np.asarray(wk, dtype=np.float32)
    wv = np.asarray(wv, dtype=np.float32)
    wo = np.asarray(wo, dtype=np.float32)
    sincos = np.asarray(sincos, dtype=np.float32)
    cm = np.asarray(causal_mask)
    sp = min(max(int(start_pos), 0), MS - S)

    tril = np.tril(np.ones((S, MS), dtype=bool))
    if not np.array_equal(cm[0, sp:sp + S, :], tril[:, :MS]):
        return _host_reference(x, wq, wk, wv, wo, sincos, start_pos,
                               cm).astype(np.float32)

    # host prep
    sc = sincos[sp:sp + S]
    sinT = np.ascontiguousarray(sc[:, :HD].T).astype(BF16NP)   # [HD, S]
    cosT = np.ascontiguousarray(sc[:, HD:].T).astype(BF16NP)   # [HD, S]
    xt = np.ascontiguousarray(x.reshape(BS, D).T).astype(BF16NP)
    wqs = (wq * np.float32(SCALE)).astype(BF16NP)
    wo_b = wo.astype(BF16NP)

    # triangular causal bias for the diagonal 128x128 sub-block
    jj = np.arange(128)[:, None]
    qq = np.arange(128)[None, :]
    maskb = np.where(jj > qq, np.float32(-30000.0),
                     np.float32(0.0)).astype(BF16NP)

    rotm = np.zeros((HD, HD), dtype=np.float32)
    hh = HD // 2
    rotm[np.arange(hh) + hh, np.arange(hh)] = -1.0
    rotm[np.arange(hh), np.arange(hh) + hh] = 1.0

    ident = np.eye(128, dtype=np.float32).astype(BF16NP)
    ones128 = np.ones((128, 128), dtype=np.float32).astype(BF16NP)

    in_maps = []
    for c in range(N_CORES):
        in_maps.append({
            "xt": xt,
            "wq": np.ascontiguousarray(wqs[:, c * QF:(c + 1) * QF]),
            "wk": wk[:, c * HD:(c + 1) * HD].astype(BF16NP),
            "wv": wv[:, c * HD:(c + 1) * HD].astype(BF16NP),
            "wo": wo_b,
            "cosT": cosT, "sinT": sinT,
            "maskb": maskb, "rotm": rotm.astype(BF16NP), "ident": ident,
            "ones128": ones128,
        })

    global _LAST_IN_MAPS
    _LAST_IN_MAPS = in_maps
    nc = _get_nc()
    res = run_bass_kernel_spmd(nc, in_maps, list(range(N_CORES)))
    # per-core y rows: [0:256] = b0 tokens c*256..; [256:384] = b1 tokens
    # c*128..; [384:512] = b1 tokens 1024+c*128..
    y = np.empty((BS, D), dtype=np.float32)
    for c in range(N_CORES):
        yc = res.results[c]["y"]
        y[c * 256:(c + 1) * 256] = yc[:256]
        y[S + c * 128:S + (c + 1) * 128] = yc[256:384]
        y[S + 1024 + c * 128:S + 1024 + (c + 1) * 128] = yc[384:]
    return y.reshape(B, S, D)


# revision 45
# speedup vs baseline: 1.0179x; 1.0179x over previous
"""Trainium2 Bass kernel for nn_Attention_13864154431876.

Dense transformer attention block: QKV projection + RoPE + causal GQA
attention (32 q heads, 8 kv heads, head_dim 128) + output projection.
B=2, S=2048, D=4096, start_pos=0 (cache fully overwritten).

Sharding (8 NeuronCores, tensor parallel by attention heads):
  - each core owns 4 q-heads and 1 kv-head (wq/wk/wv output-dim shards)
  - x is replicated (shipped pre-transposed as x^T so the contraction dim
    lands on partitions)
  - after attention, an on-chip AllToAll redistributes attn^T from
    head-sharded to token-sharded; each core then multiplies its 512-token
    slab against the full wo and the host concatenates the 8 slabs.

All on-chip data is bf16 (PSUM accumulation stays fp32); the 2e-2
relative-error budget has ample headroom for it, it halves all HBM/DMA
traffic and doubles DVE elementwise throughput.

Attention emission is software-pipelined: the AV matmul for chunk jc
is emitted one step after its score matmul so the exp (scalar engine)
latency never stalls the in-order tensor engine. The causal mask is
applied as a bias-accumulate matmul on the tensor engine (a triangular
-30000 tile added to the diagonal 128x128 sub-block of the scores) and
diagonal score/AV matmuls are shrunk to the live query subrange.
"""
import sys

sys.path.insert(0, "/root/.axon_site/_ro/trn_rl_repo")

import numpy as np
import ml_dtypes

import concourse.bass as bass
import concourse.mybir as mybir
import concourse.tile as tile
from concourse import bacc
from concourse.bass_utils import run_bass_kernel_spmd

F32 = mybir.dt.float32
BF16 = mybir.dt.bfloat16
AF = mybir.ActivationFunctionType
ALU = mybir.AluOpType

N_CORES = 8
B, S, D = 2, 2048, 4096
H, KH, HD = 32, 8, 128
MS = 2048                     # max_seq_len (cache length)
BS = B * S                    # flattened tokens, b-major
HPC = H // N_CORES            # q-heads per core = 4
QF = HPC * HD                 # per-core q-feature width = 512
TB = 512                      # token block
NTB = BS // TB                # 8 token blocks
QBPB = S // TB                # 4 q-blocks per batch element
KC = D // 128                 # 32 contraction chunks
JCB = S // 128                # 16 j-chunks per batch element
SCALE = 1.0 / np.sqrt(HD)
TOKS_PER_CORE = BS // N_CORES  # 512

BF16NP = ml_dtypes.bfloat16


def build_attn_nc(mock_collectives=False):
    nc = bacc.Bacc("TRN2", target_bir_lowering=False, debug=False,
                   num_devices=N_CORES)

    # ---- DRAM I/O ----------------------------------------------------
    xt_d = nc.dram_tensor("xt", [D, BS], BF16, kind="ExternalInput").ap()
    wq_d = nc.dram_tensor("wq", [D, QF], BF16, kind="ExternalInput").ap()
    wk_d = nc.dram_tensor("wk", [D, HD], BF16, kind="ExternalInput").ap()
    wv_d = nc.dram_tensor("wv", [D, HD], BF16, kind="ExternalInput").ap()
    wo_d = nc.dram_tensor("wo", [D, D], BF16, kind="ExternalInput").ap()
    cos_d = nc.dram_tensor("cosT", [HD, S], BF16, kind="ExternalInput").ap()
    sin_d = nc.dram_tensor("sinT", [HD, S], BF16, kind="ExternalInput").ap()
    maskb_d = nc.dram_tensor("maskb", [128, 128], BF16, kind="ExternalInput").ap()
    rot_d = nc.dram_tensor("rotm", [HD, HD], BF16, kind="ExternalInput").ap()
    ident_d = nc.dram_tensor("ident", [128, 128], BF16, kind="ExternalInput").ap()
    ones_d = nc.dram_tensor("ones128", [128, 128], BF16, kind="ExternalInput").ap()
    y_d = nc.dram_tensor("y", [TOKS_PER_CORE, D], F32, kind="ExternalOutput").ap()

    # internal DRAM for the per-batch-element AllToAlls.
    # attn_locX rows are chunk-major: chunk j (512 rows) = my 512 head-feats
    # for token group j. After A2A, attn_gX rows are global head-feats for
    # MY token slab.
    HTB = TB // 2  # 256
    QTB = TB // 4  # 128
    attn_loc = [nc.dram_tensor("attn_loc0", [BS, HTB], BF16),
                nc.dram_tensor("attn_loc1a", [BS, QTB], BF16),
                nc.dram_tensor("attn_loc1b", [BS, QTB], BF16)]
    attn_g = [nc.dram_tensor("attn_g0", [D, HTB], BF16),
              nc.dram_tensor("attn_g1a", [D, QTB], BF16),
              nc.dram_tensor("attn_g1b", [D, QTB], BF16)]

    with tile.TileContext(nc) as tc:
        # ---- pool stack (bottom-up; closed LIFO) --------------------
        persist0_cm = tc.tile_pool(name="persist0", bufs=1)
        persist0 = persist0_cm.__enter__()
        maskb_sb = persist0.tile([128, 128], BF16, name="maskb_sb")
        rot_sb = persist0.tile([HD, HD], BF16, name="rot_sb")
        ident_sb = persist0.tile([128, 128], BF16, name="ident_sb")
        ones_sb = persist0.tile([128, 128], BF16, name="ones_sb")
        kt_sb = [persist0.tile([HD, S], BF16, name=f"kt{b}_sb") for b in range(B)]
        v_sb = [persist0.tile([128, JCB, HD], BF16, name=f"v{b}_sb")
                for b in range(B)]
        attn_sb = [persist0.tile([128, KC, 128], BF16, name=f"attn_sb{i}")
                   for i in range(4)]

        # wo(0) ring: slots exist from the start; the DMAs are emitted just
        # before the tail so the load runs under the tail attention.
        wop_cm = tc.tile_pool(name="wop", bufs=8)
        wop = wop_cm.__enter__()

        tier2 = []

        def pool_t2(*a, **kw):
            cm = tc.tile_pool(*a, **kw)
            p = cm.__enter__()
            tier2.append(cm)
            return p

        qtp = pool_t2(name="qtp", bufs=5)
        tmpp = pool_t2(name="tmpp", bufs=2)
        ptp = pool_t2(name="ptp", bufs=4)
        denp = pool_t2(name="denp", bufs=2)
        recbp = pool_t2(name="recbp", bufs=2)
        atp = pool_t2(name="atp", bufs=2)
        asbp = pool_t2(name="asbp", bufs=2)

        persistA_cm = tc.tile_pool(name="persistA", bufs=1)
        persistA = persistA_cm.__enter__()
        wq_sb = persistA.tile([128, KC, QF], BF16, name="wq_sb")
        wk_sb = persistA.tile([128, KC, HD], BF16, name="wk_sb")
        wv_sb = persistA.tile([128, KC, HD], BF16, name="wv_sb")
        cos_sb = persistA.tile([HD, S], BF16, name="cos_sb")
        sin_sb = persistA.tile([HD, S], BF16, name="sin_sb")
        # weight/constant prefetch rides the scalar-engine DMA queue so it
        # never head-of-line-blocks the latency-critical xt stream (sync
        # queue); chunked so the first projection matmuls start immediately.
        wq_r = wq_d.rearrange("(kc p) n -> p kc n", p=128)
        wk_r = wk_d.rearrange("(kc p) n -> p kc n", p=128)
        wv_r = wv_d.rearrange("(kc p) n -> p kc n", p=128)
        def load_w_group(g4):
            lo, hi = g4 * 4, g4 * 4 + 4
            nc.sync.dma_start(wq_sb[:, lo:hi, :], wq_r[:, lo:hi, :])
            nc.sync.dma_start(wk_sb[:, lo:hi, :], wk_r[:, lo:hi, :])
            nc.sync.dma_start(wv_sb[:, lo:hi, :], wv_r[:, lo:hi, :])

        # weights ride the sync queue interleaved with block 0's xt chunks
        # (emitted inside the kc loop) so FIFO order gives them priority
        # over the deep xt prefetch burst
        load_w_group(0)
        nc.scalar.dma_start(rot_sb[:], rot_d[:])
        nc.scalar.dma_start(ident_sb[:], ident_d[:])
        nc.scalar.dma_start(cos_sb[:], cos_d[:])
        nc.scalar.dma_start(sin_sb[:], sin_d[:])
        nc.scalar.dma_start(maskb_sb[:], maskb_d[:])
        nc.scalar.dma_start(ones_sb[:], ones_d[:])

        trans = []

        def pool_tr(*a, **kw):
            cm = tc.tile_pool(*a, **kw)
            p = cm.__enter__()
            trans.append(cm)
            return p

        # deep ring: the A2A collectives steal DMA bandwidth for ~30us
        # bursts; 12 chunks of lookahead lets the xt stream ride through
        xtp = pool_tr(name="xtp", bufs=12)
        qrawp = pool_tr(name="qrawp", bufs=6)
        vtrawp = pool_tr(name="vtrawp", bufs=2)
        pp_cm = tc.tile_pool(name="pp", bufs=6, space="PSUM")
        pp = pp_cm.__enter__()
        ps_cm = tc.tile_pool(name="ps", bufs=1, space="PSUM")
        ps = ps_cm.__enter__()
        pa_cm = tc.tile_pool(name="pa", bufs=1, space="PSUM")
        pa = pa_cm.__enter__()

        def emit_attention(tb, ps_pool, pa_pool, qt_tiles, lag=1):
            """Generator: pipelined attention for token block tb.

            Per step: score(jc) [+mask bias], exp(jc), den(jc), then the AV
            matmul for chunk jc-lag — so the tensor engine never waits on
            the scalar-engine exp chain."""
            b, qb = tb // QBPB, tb % QBPB
            njc = (qb + 1) * 4
            grp = 0 if tb < QBPB else (1 if tb < 6 else 2)
            for h in range(HPC):
                denacc = denp.tile([128, TB], BF16, name="denacc", tag="den")
                aps = pa_pool.tile([128, TB], F32, name="aps", tag="aps")
                pending = []

                def emit_av(jc, pt, lo):
                    nc.tensor.matmul(
                        aps[:, lo:], v_sb[b][:, jc, :], pt[:, lo:],
                        start=(jc == 0), stop=(jc == njc - 1),
                        skip_group_check=True)

                for jc in range(njc):
                    r = jc - qb * 4
                    lo = max(r, 0) * 128
                    sps = ps_pool.tile([128, TB], F32, name="sps", tag="sps")
                    nc.tensor.matmul(
                        sps[:, lo:], kt_sb[b][:, jc * 128:(jc + 1) * 128],
                        qt_tiles[h][:, lo:], start=True, stop=(r < 0),
                        skip_group_check=True)
                    if r >= 0:
                        # causal bias: triangular -30000 on the diagonal
                        # 128x128 sub-block, accumulated on the PE
                        nc.tensor.matmul(
                            sps[:, lo:lo + 128], ident_sb[:], maskb_sb[:],
                            start=False, stop=True, skip_group_check=True)
                    pt = ptp.tile([128, TB], BF16, name="pt", tag="pt")
                    nc.scalar.activation(pt[:, lo:], sps[:, lo:], AF.Exp)
                    if jc == 0:
                        nc.vector.tensor_copy(denacc[:], pt[:])
                    else:
                        nc.vector.tensor_tensor(denacc[:, lo:], denacc[:, lo:],
                                                pt[:, lo:], ALU.add)
                    pending.append((jc, pt, lo))
                    while len(pending) > lag:
                        emit_av(*pending.pop(0))
                    yield
                while pending:
                    emit_av(*pending.pop(0))
                    yield
                # epilogue: colsum+broadcast via ones-matmul, fast
                # reciprocal, normalize; aps drained by the scalar engine
                asb = asbp.tile([128, TB], BF16, name="asb", tag="asb")
                nc.scalar.copy(asb[:], aps[:])
                denb = ps_pool.tile([128, TB], F32, name="denb", tag="sps")
                nc.tensor.matmul(denb[:], ones_sb[:], denacc[:],
                                 start=True, stop=True, skip_group_check=True)
                recipb = recbp.tile([128, TB], F32, name="recipb", tag="recb")
                nc.vector.reciprocal_approx_fast(recipb[:], denb[:])
                yield
                attn_t = atp.tile([128, TB], BF16, name="attn_t", tag="attn_t")
                nc.vector.tensor_tensor(attn_t[:], asb[:], recipb[:], ALU.mult)
                if grp == 0:
                    for half in range(2):
                        nc.sync.dma_start(
                            attn_loc[0].ap()[
                                (2 * tb + half) * 512 + h * 128:
                                (2 * tb + half) * 512 + (h + 1) * 128, :],
                            attn_t[:, half * HTB:(half + 1) * HTB])
                else:
                    lb = (tb - 4) % 2
                    for qt4 in range(4):
                        nc.sync.dma_start(
                            attn_loc[grp].ap()[
                                (4 * lb + qt4) * 512 + h * 128:
                                (4 * lb + qt4) * 512 + (h + 1) * 128, :],
                            attn_t[:, qt4 * QTB:(qt4 + 1) * QTB])
                yield

        def drive(gen, n):
            if gen is None:
                return None
            for _ in range(n):
                try:
                    next(gen)
                except StopIteration:
                    return None
            return gen

        def emit_a2a(g):
            if mock_collectives:
                nc.sync.dma_start(attn_g[g].ap()[:], attn_loc[g].ap()[:])
            else:
                nc.gpsimd.collective_compute(
                    "AllToAll", ALU.bypass,
                    replica_groups=[list(range(N_CORES))],
                    ins=[attn_loc[g].ap().opt()],
                    outs=[attn_g[g].ap().opt()],
                )

        def load_attn_sb(i):
            if i < 2:
                src = attn_g[0].ap().rearrange("(hc p) q -> p hc q", p=128)[
                    :, :, i * 128:(i + 1) * 128]
            else:
                src = attn_g[i - 1].ap().rearrange("(hc p) q -> p hc q", p=128)
            # SWDGE on the idle Pool engine: these loads depend on the
            # collectives, and on a HWDGE queue they would head-of-line
            # block the latency-critical xt/epilogue/y stream.
            nc.gpsimd.dma_start(attn_sb[i][:], src)

        wo_r = wo_d.rearrange("(hc p) n -> p hc n", p=128)

        def load_wo(ob, pool=None):
            tiles = []
            for g in range(8):
                wt = (pool or wop).tile([128, 4, TB], BF16, name="wo_t",
                                        tag="wo")
                nc.sync.dma_start(
                    wt[:], wo_r[:, g * 4:(g + 1) * 4, ob * TB:(ob + 1) * TB])
                tiles.append(wt)
            return tiles

        prev_gen = None
        prev_steps = 0
        for tb in range(NTB):
            b, qb = tb // QBPB, tb % QBPB
            s0 = qb * TB
            # fractional pacing: spread attention(tb-1)'s steps over the
            # 32 kc iterations AND the drain section so the tensor engine
            # is never left without interleaved work
            pace_slots = KC + 5
            pace_acc = 0
            if tb == NTB - 1:
                # both A2As are long done; load early so the attn_sb tiles
                # are resident before the tail/phase-3 interleave begins
                load_attn_sb(0)
                load_attn_sb(1)
                load_attn_sb(2)
            # ---- projections for tb, interleaved with attention(tb-1)
            qps = [pp.tile([128, TB], F32, name=f"qps{h}", tag="proj")
                   for h in range(HPC)]
            kps = pp.tile([128, TB], F32, name="kps", tag="proj")
            vtps = pp.tile([128, TB], F32, name="vtps", tag="proj")
            for kc in range(KC):
                xt_t = xtp.tile([128, TB], BF16, name="xt_t", tag="xt")
                nc.sync.dma_start(
                    xt_t[:], xt_d[kc * 128:(kc + 1) * 128,
                                  tb * TB:(tb + 1) * TB])
                for h in range(HPC):
                    nc.tensor.matmul(
                        qps[h][:], wq_sb[:, kc, h * 128:(h + 1) * 128],
                        xt_t[:], start=(kc == 0), stop=(kc == KC - 1),
                        skip_group_check=True)
                nc.tensor.matmul(kps[:], wk_sb[:, kc, :], xt_t[:],
                                 start=(kc == 0), stop=(kc == KC - 1),
                                 skip_group_check=True)
                nc.tensor.matmul(vtps[:], wv_sb[:, kc, :], xt_t[:],
                                 start=(kc == 0), stop=(kc == KC - 1),
                                 skip_group_check=True)
                if tb == 0 and kc in (1, 4, 8, 12, 16, 20, 24):
                    load_w_group(kc // 4 + 1)
                tgt = (kc + 1) * prev_steps // pace_slots
                prev_gen = drive(prev_gen, tgt - pace_acc)
                pace_acc = tgt

            # ---- drains + RoPE + V transpose ------------------------
            # All PSUM->SBUF drains go out on the scalar engine first;
            # the rot matmuls then land in the just-freed qps/kps/vtps
            # PSUM banks (pp pool rotation lines up 1:1) so nothing
            # ping-pongs on the attention score bank. K and V complete
            # first so this block's attention can start as soon as its
            # first q head is rotated — filling the drain section when
            # the previous block's attention has run dry.
            kraw = qrawp.tile([128, TB], BF16, name="kraw", tag="qraw")
            nc.scalar.copy(kraw[:], kps[:])
            vtraw = vtrawp.tile([128, TB], BF16, name="vtraw", tag="vtraw")
            nc.scalar.copy(vtraw[:], vtps[:])
            qraws = []
            for h in range(HPC):
                qraw = qrawp.tile([128, TB], BF16, name="qraw", tag="qraw")
                nc.scalar.copy(qraw[:], qps[h][:])
                qraws.append(qraw)
            # K
            rotps = pp.tile([128, TB], F32, name="rotpsk", tag="proj")
            nc.tensor.matmul(rotps[:], rot_sb[:], kraw[:],
                             start=True, stop=True, skip_group_check=True)
            tcos = tmpp.tile([128, TB], BF16, name="tcosk", tag="tmp")
            nc.vector.tensor_tensor(tcos[:], kraw[:],
                                    cos_sb[:, s0:s0 + TB], ALU.mult)
            tsin = tmpp.tile([128, TB], BF16, name="tsink", tag="tmp")
            nc.vector.tensor_tensor(tsin[:], rotps[:],
                                    sin_sb[:, s0:s0 + TB], ALU.mult)
            nc.vector.tensor_tensor(kt_sb[b][:, s0:s0 + TB], tcos[:],
                                    tsin[:], ALU.add)
            # V: transpose 4x [128,128]
            vtr = pp.tile([128, TB], BF16, name="vtr", tag="proj")
            for t4 in range(4):
                nc.tensor.transpose(vtr[:, t4 * 128:(t4 + 1) * 128],
                                    vtraw[:, t4 * 128:(t4 + 1) * 128],
                                    ident_sb[:])
            nc.vector.tensor_copy(
                v_sb[b].rearrange("p jc d -> p (jc d)")[:, s0:s0 + TB],
                vtr[:])
            # Q heads + early start of this block's attention
            qt_tiles = []
            cur_gen = None
            early_steps = 0
            for h in range(HPC):
                rotps = pp.tile([128, TB], F32, name="rotps", tag="proj")
                nc.tensor.matmul(rotps[:], rot_sb[:], qraws[h][:],
                                 start=True, stop=True, skip_group_check=True)
                tcos = tmpp.tile([128, TB], BF16, name="tcos", tag="tmp")
                nc.vector.tensor_tensor(tcos[:], qraws[h][:],
                                        cos_sb[:, s0:s0 + TB], ALU.mult)
                tsin = tmpp.tile([128, TB], BF16, name="tsin", tag="tmp")
                nc.vector.tensor_tensor(tsin[:], rotps[:],
                                        sin_sb[:, s0:s0 + TB], ALU.mult)
                qt = qtp.tile([128, TB], BF16, name="qt", tag="qt")
                nc.vector.tensor_tensor(qt[:], tcos[:], tsin[:], ALU.add)
                qt_tiles.append(qt)
                if tb < NTB - 1 and h == 0:
                    cur_gen = emit_attention(tb, ps, pa, qt_tiles)
                tgt = (KC + 1 + h) * prev_steps // pace_slots
                prev_gen = drive(prev_gen, tgt - pace_acc)
                pace_acc = tgt
            prev_gen = drive(prev_gen, 10 ** 9)  # flush any leftovers
            if tb == 0:
                # block 0 has no previous attention to interleave; fill the
                # drain hole with its own first attention steps (inputs —
                # kt, v, qt0/qt1 — are all complete by this point)
                cur_gen = drive(cur_gen, 8)
                early_steps = 8
            # A2As fire as soon as their group's epilogue DMAs land; the
            # attn_sb loads are EMITTED a full block later so their DMA-
            # completion semaphores (which alias with the xt stream's)
            # never transitively gate projection matmuls on the collective.
            if tb == 4:
                emit_a2a(0)
            elif tb == 6:
                emit_a2a(1)
            if tb < NTB - 1:
                prev_gen = cur_gen
                prev_steps = max(1, HPC * ((qb + 1) * 4 + 3) - early_steps)
            else:
                tail_qt = qt_tiles

        # ---- free projection-only pools; prefetch wo(ob=0) ----------
        wo_next = load_wo(0)
        pa_cm.__exit__(None, None, None)
        ps_cm.__exit__(None, None, None)
        pp_cm.__exit__(None, None, None)
        for cm in reversed(trans):
            cm.__exit__(None, None, None)
        persistA_cm.__exit__(None, None, None)

        # ---- attention tail (tb=7), interleaved with ob=0's output
        # matmuls (dependency-free: attn_sb[0..2] + wo(0) are resident),
        # which fill the tensor-engine bubbles of the exp-bound tail ----
        py_cm = tc.tile_pool(name="py", bufs=4, space="PSUM")
        pyp = py_cm.__enter__()
        ps2_cm = tc.tile_pool(name="ps2", bufs=3, space="PSUM")
        ps2 = ps2_cm.__enter__()
        pa2_cm = tc.tile_pool(name="pa2", bufs=1, space="PSUM")
        pa2 = pa2_cm.__enter__()

        def ygroup_gen(ob, tc4, wo_g):
            yps = pyp.tile([128, TB], F32, name="yps", tag="yps")
            for hc in range(KC):
                nc.tensor.matmul(
                    yps[:], attn_sb[tc4][:, hc, :],
                    wo_g[hc // 4][:, hc % 4, :],
                    start=(hc == 0), stop=(hc == KC - 1),
                    skip_group_check=True)
                if hc % 4 == 3:
                    yield
            y_sb = persist0.tile([128, TB], F32, name="y_sb", tag="y",
                                 bufs=6)
            nc.vector.tensor_copy(y_sb[:], yps[:])
            nc.sync.dma_start(
                y_d[tc4 * 128:(tc4 + 1) * 128,
                    ob * TB:(ob + 1) * TB], y_sb[:])

        def emit_ygroup(ob, tc4, wo_g):
            drive(ygroup_gen(ob, tc4, wo_g), 10 ** 9)

        def chain_gens(gens):
            for g in gens:
                yield from g

        tail_gen = emit_attention(NTB - 1, ps2, pa2, tail_qt, lag=2)
        ygen = chain_gens([ygroup_gen(0, t, wo_next) for t in range(3)])
        cnt = 0
        while tail_gen is not None:
            tail_gen = drive(tail_gen, 1)
            cnt += 1
            # start a few steps in so the wo(0) stream has landed
            if cnt % 3 == 0 and cnt >= 12:
                ygen = drive(ygen, 1)
        emit_a2a(2)
        ygen = drive(ygen, 10 ** 9)
        pa2_cm.__exit__(None, None, None)
        ps2_cm.__exit__(None, None, None)
        for cm in reversed(tier2):
            cm.__exit__(None, None, None)

        wop2_cm = tc.tile_pool(name="wop2", bufs=24)
        wop2 = wop2_cm.__enter__()

        # ---- phase 3 remainder --------------------------------------
        # The tc4=3 groups need attn_sb[3] which arrives only after the
        # final A2A (~30us) — so ob1/ob2's tc4 0..2 groups (~40us of
        # matmuls) are emitted first as cover, with the t3 load deferred
        # past them so its DMA semaphore gates nothing early.
        wo_t = {0: wo_next}
        wo_t[1] = load_wo(1, wop2)
        wo_t[2] = load_wo(2, wop2)
        wo_t[3] = load_wo(3, wop2)
        for tc4 in range(3):
            emit_ygroup(1, tc4, wo_t[1])
        for tc4 in range(3):
            emit_ygroup(2, tc4, wo_t[2])
        load_attn_sb(3)
        emit_ygroup(0, 3, wo_t[0])
        emit_ygroup(1, 3, wo_t[1])
        wo_t[4] = load_wo(4, wop2)
        emit_ygroup(2, 3, wo_t[2])
        wo_t[5] = load_wo(5, wop2)
        for tc4 in range(4):
            emit_ygroup(3, tc4, wo_t[3])
        wo_t[6] = load_wo(6, wop2)
        for tc4 in range(4):
            emit_ygroup(4, tc4, wo_t[4])
        wo_t[7] = load_wo(7, wop2)
        for ob in range(5, 8):
            for tc4 in range(4):
                if ob == 7 and tc4 == 3:
                    continue
                emit_ygroup(ob, tc4, wo_t[ob])
        # final group in two column halves so the first half's drain and
        # store overlap the second half's matmuls
        for half in range(2):
            yps = pyp.tile([128, HTB], F32, name="ypsh", tag="yps")
            for hc in range(KC):
                nc.tensor.matmul(
                    yps[:], attn_sb[3][:, hc, :],
                    wo_t[7][hc // 4][:, hc % 4,
                                     half * HTB:(half + 1) * HTB],
                    start=(hc == 0), stop=(hc == KC - 1),
                    skip_group_check=True)
            y_sb = persist0.tile([128, HTB], F32, name="y_sbh", tag="yh",
                                 bufs=2)
            nc.vector.tensor_copy(y_sb[:], yps[:])
            nc.sync.dma_start(
                y_d[3 * 128:4 * 128,
                    7 * TB + half * HTB:7 * TB + (half + 1) * HTB], y_sb[:])
        py_cm.__exit__(None, None, None)
        wop2_cm.__exit__(None, None, None)
        wop_cm.__exit__(None, None, None)
        persist0_cm.__exit__(None, None, None)

    nc.compile()
    return nc


_NC_CACHE = None


def _get_nc():
    global _NC_CACHE
    if _NC_CACHE is None:
        _NC_CACHE = build_attn_nc()
    return _NC_CACHE


def _host_reference(x, wq, wk, wv, wo, sincos, start_pos, causal_mask):
    """Numpy fallback (only used if the mask is not causal-tril)."""
    xq = (x @ wq).reshape(B, S, H, HD)
    xk = (x @ wk).reshape(B, S, KH, HD)
    xv = (x @ wv).reshape(B, S, KH, HD)
    sp = min(max(int(start_pos), 0), MS - S)
    sc = sincos[sp:sp + S]
    sin, cos = sc[:, :HD], sc[:, HD:]
    sin = sin[None, :, None, :]
    cos = cos[None, :, None, :]

    def rot(u):
        return np.concatenate([-u[..., HD // 2:], u[..., :HD // 2]], axis=-1)

    xq = xq * cos + rot(xq) * sin
    xk = xk * cos + rot(xk) * sin
    mask = np.broadcast_to(causal_mask[:, sp:sp + S, :MS], (B, S, MS))
    out = np.zeros((B, S, H, HD), dtype=np.float32)
    nrep = H // KH
    for b in range(B):
        for h in range(H):
            q = xq[b, :, h]
            k = xk[b, :, h // nrep]
            v = xv[b, :, h // nrep]
            s = (q @ k.T) * SCALE
            s = np.where(mask[b], s, -np.inf)
            s = s - s.max(axis=-1, keepdims=True)
            p = np.exp(s)
            p /= p.sum(axis=-1, keepdims=True)
            out[b, :, h] = p @ v
    return out.reshape(B, S, H * HD) @ wo


def kernel(x, wq, wk, wv, wo, cache_k, cache_v, sincos, causal_mask,
           start_pos):
    x = np.asarray(x, dtype=np.float32)
    wq = np.asarray(wq, dtype=np.float32)
    wk = np.asarray(wk, dtype=np.float32)
    wv = np.asarray(wv, dtype=np.float32)
    wo = np.asarray(wo, dtype=np.float32)
    sincos = np.asarray(sincos, dtype=np.float32)
    cm = np.asarray(causal_mask)
    sp = min(max(int(start_pos), 0), MS - S)

    tril = np.tril(np.ones((S, MS), dtype=bool))
    if not np.array_equal(cm[0, sp:sp + S, :], tril[:, :MS]):
        return _host_reference(x, wq, wk, wv, wo, sincos, start_pos,
                               cm).astype(np.float32)

    # host prep
    sc = sincos[sp:sp + S]
    sinT = np.ascontiguousarray(sc[:, :HD].T).astype(BF16NP)   # [HD, S]
    cosT = np.ascontiguousarray(sc[:, HD:].T).astype(BF16NP)   # [HD, S]
    xt = np.ascontiguousarray(x.reshape(BS, D).T).astype(BF16NP)
    wqs = (wq * np.float32(SCALE)).astype(BF16NP)
    wo_b = wo.astype(BF16NP)

    # triangular causal bias for the diagonal 128x128 sub-block
    jj = np.arange(128)[:, None]
    qq = np.arange(128)[None, :]
    maskb = np.where(jj > qq, np.float32(-30000.0),
                     np.float32(0.0)).astype(BF16NP)

    rotm = np.zeros((HD, HD), dtype=np.float32)
    hh = HD // 2
    rotm[np.arange(hh) + hh, np.arange(hh)] = -1.0
    rotm[np.arange(hh), np.arange(hh) + hh] = 1.0

    ident = np.eye(128, dtype=np.float32).astype(BF16NP)
    ones128 = np.ones((128, 128), dtype=np.float32).astype(BF16NP)

    in_maps = []
    for c in range(N_CORES):
        in_maps.append({
            "xt": xt,
            "wq": np.ascontiguousarray(wqs[:, c * QF:(c + 1) * QF]),
            "wk": wk[:, c * HD:(c + 1) * HD].astype(BF16NP),
            "wv": wv[:, c * HD:(c + 1) * HD].astype(BF16NP),
            "wo": wo_b,
            "cosT": cosT, "sinT": sinT,
            "maskb": maskb, "rotm": rotm.astype(BF16NP), "ident": ident,
            "ones128": ones128,
        })

    global _LAST_IN_MAPS
    _LAST_IN_MAPS = in_maps
    nc = _get_nc()
    res = run_bass_kernel_spmd(nc, in_maps, list(range(N_CORES)))
    # per-core y rows: [0:256] = b0 tokens c*256..; [256:384] = b1 tokens
    # c*128..; [384:512] = b1 tokens 1024+c*128..
    y = np.empty((BS, D), dtype=np.float32)
    for c in range(N_CORES):
        yc = res.results[c]["y"]
        y[c * 256:(c + 1) * 256] = yc[:256]
        y[S + c * 128:S + (c + 1) * 128] = yc[256:384]
        y[S + 1024 + c * 128:S + 1024 + (c + 1) * 128] = yc[384:]
    return y.reshape(B, S, D)
